# revision 6
# baseline (speedup 1.0000x reference)
"""Self-contained segment-max kernel for 8 TRN2 NeuronCores.

out[s, d] = max over rows i with index[i] == s of x[i, d]; empty
segments yield 0. Shapes hardcoded per the problem spec:
x [4194304, 64] f32, index [4194304] int64 (sorted), 65536 segments.

Algorithm (output-stationary chunked gather, bf16):
  * Host: casts x to bf16 (tolerance 2e-2 >> bf16's 2^-9 rounding) —
    halves HBM traffic. From the sorted index, bincount/cumsum give
    per-segment row ranges. Segments are split into 8 contiguous ranges
    with ~equal row counts; segments never straddle cores, so there is
    no cross-core combine.
  * dma_gather's index stride unit must be a multiple of 256B = 2 bf16
    rows, so the host stages each core's slab in an ALIGNED layout:
    every segment starts at an even row and is padded to even length by
    duplicating its last row (~+1% rows; duplicates are free for max).
    Gather indices address 256B row-PAIRS, so one int16 window spans
    65024 rows (W_PAIRS=32512 pair stride, 32768-pair AP extent leaves
    slack for segments starting near the window end).
  * Within a window, segments are sorted by length (desc) and packed
    into tiles of 128 (one segment per SBUF partition). Each segment is
    covered by ITER chunks of C consecutive rows (C even), chunk j
    starting at clamp(a + j*C, a, b-C) — all even. C <= min length in
    the tile, so chunks never cross segment boundaries.
  * Device: per tile-job, dma_gather (elem_step=128 elems = one 256B
    pair, elem_size=C*64) pulls 128*ITER chunks into an SBUF tile
    [128, ITER*C*64] bf16; the per-partition max over rows is computed
    with a fold-halves tensor_tensor(max) tree (contiguous packed bf16
    operands run in the DVE 2x_1p mode, 2x the tensor_reduce rate; odd
    row counts overlap the two halves — duplicates are free for max),
    finishing with one small strided tensor_reduce; [128, 64] bf16 goes
    to the job's output slot. Gather calls are capped at 8*128 indices:
    the per-lane SWDGE descriptor ring holds ~65 descriptors and one
    call generates num_idxs/16+1 per lane.
  * Job constants (window, C, ITER) are maxed across cores so a single
    SPMD NEFF serves all 8 cores; all per-core variation lives in the
    gather-index input tensor. Host scatters per-slot results back to
    segment ids (cast to f32) and leaves empty segments at 0.

The result is exact max over bf16-rounded inputs (max is order- and
duplicate-invariant), rel err <= 2^-8 vs f32.
"""

import os
import sys

sys.path.insert(0, "/opt/trn_rl_repo")

import numpy as np
import ml_dtypes

BF16 = ml_dtypes.bfloat16

N_FULL = 4194304
NUM_SEGMENTS = 65536
D = 64
N_CORES = 8
W_PAIRS = 32512          # window stride in 256B row-pairs
NW_PAIRS = 32768         # gather AP extent in pairs
W_ROWS = 2 * W_PAIRS
P = 128
C_DEFAULT = 16
MAX_IT_PER_CALL = 8
FOLD_STOP = 8  # switch from fold tree to one strided tensor_reduce

LAST_RUN_INFO = {}


def _plan(index, c_default=C_DEFAULT):
    n = index.shape[0]
    counts = np.bincount(index, minlength=NUM_SEGMENTS).astype(np.int64)
    starts = np.zeros(NUM_SEGMENTS + 1, dtype=np.int64)
    np.cumsum(counts, out=starts[1:])
    assert starts[-1] == n

    seg_bounds = np.searchsorted(
        starts, [n * c // N_CORES for c in range(N_CORES + 1)]
    )
    seg_bounds[0] = 0
    seg_bounds[-1] = NUM_SEGMENTS

    core_segs = []      # per core: ids of non-empty segments (sorted)
    core_astarts = []   # per core: aligned start row of each segment
    core_alens = []     # per core: aligned (even) length of each segment
    core_arows = []     # per core: total aligned rows
    for c in range(N_CORES):
        s0, s1 = seg_bounds[c], seg_bounds[c + 1]
        segs = np.arange(s0, s1)[counts[s0:s1] > 0]
        lens = counts[segs]
        alens = lens + (lens & 1)
        astarts = np.zeros(len(segs) + 1, dtype=np.int64)
        np.cumsum(alens, out=astarts[1:])
        core_segs.append(segs)
        core_astarts.append(astarts)
        core_alens.append(alens)
        core_arows.append(int(astarts[-1]))
        assert alens.max(initial=0) <= 512, "segment too long for window slack"

    max_rows = max(core_arows)
    n_windows = max(1, -(-max_rows // W_ROWS))
    ns = (n_windows - 1) * W_ROWS + 2 * NW_PAIRS + 512

    # per core, per window: positional indices into core_segs, sorted by
    # aligned length desc
    win_ord = [[None] * n_windows for _ in range(N_CORES)]
    for c in range(N_CORES):
        w_of = core_astarts[c][:-1] // W_ROWS
        for w in range(n_windows):
            pos = np.nonzero(w_of == w)[0]
            order = np.argsort(-core_alens[c][pos], kind="stable")
            win_ord[c][w] = pos[order]

    jobs = []  # (window, tile, C, ITER) shared across all cores
    for w in range(n_windows):
        t_w = max(-(-len(win_ord[c][w]) // P) for c in range(N_CORES))
        for t in range(t_w):
            minlen = c_default
            maxlen = 2
            for c in range(N_CORES):
                sl = win_ord[c][w][t * P : (t + 1) * P]
                if len(sl):
                    minlen = min(minlen, int(core_alens[c][sl].min()))
                    maxlen = max(maxlen, int(core_alens[c][sl].max()))
            cc = max(2, 2 * (min(c_default, minlen) // 2))
            it = max(1, -(-maxlen // cc))
            jobs.append((w, t, cc, it))

    njobs = len(jobs)
    idxw = sum(8 * it for (_, _, _, it) in jobs)

    gidx = np.zeros((N_CORES, P, idxw), dtype=np.int16)
    slotseg = np.full((N_CORES, njobs * P), -1, dtype=np.int64)
    for c in range(N_CORES):
        astarts, alens, segs = core_astarts[c], core_alens[c], core_segs[c]
        off = 0
        for k, (w, t, cc, it) in enumerate(jobs):
            sl = win_ord[c][w][t * P : (t + 1) * P]
            nsl = len(sl)
            a = np.full(P, w * W_ROWS, dtype=np.int64)
            b = a + cc
            if nsl:
                a[:nsl] = astarts[sl]
                b[:nsl] = astarts[sl] + alens[sl]
                slotseg[c, k * P : k * P + nsl] = segs[sl]
            j = np.arange(it, dtype=np.int64)[:, None]
            st = np.minimum(a[None, :] + j * cc, b[None, :] - cc)
            st = np.maximum(st, a[None, :])
            assert (st % 2 == 0).all()
            stp = st // 2 - w * W_PAIRS
            assert stp.min() >= 0 and stp.max() < NW_PAIRS
            # flat order i = j*128 + p matches the gather's dst[p, j]
            flat = stp.astype(np.int16).reshape(-1)
            wrapped = flat.reshape(-1, 16).T  # [16, 8*it] idx stream
            gidx[c, :, off : off + 8 * it] = np.tile(wrapped, (8, 1))
            off += 8 * it

    return dict(
        ns=ns,
        idxw=idxw,
        jobs=jobs,
        njobs=njobs,
        gidx=gidx,
        slotseg=slotseg,
        seg_bounds=seg_bounds,
        starts=starts,
        counts=counts,
        core_segs=core_segs,
        core_astarts=core_astarts,
        core_alens=core_alens,
        core_arows=core_arows,
    )


def _build(pl, enable_asserts=False, reps=1, queues=1, skip_reduce=False, bufs=6):
    import concourse.bacc as bacc
    import concourse.bass as bass
    import concourse.mybir as mybir
    import concourse.tile as tile

    nc = bacc.Bacc(
        "TRN2",
        debug=False,
        enable_asserts=enable_asserts,
        target_bir_lowering=False,
        num_devices=N_CORES,
        num_swdge_queues=queues,
    )
    xs = nc.dram_tensor("xs", [pl["ns"], D], mybir.dt.bfloat16, kind="ExternalInput")
    gi = nc.dram_tensor(
        "gidx", [P, pl["idxw"]], mybir.dt.int16, kind="ExternalInput"
    )
    out = nc.dram_tensor(
        "out", [pl["njobs"] * P, D], mybir.dt.bfloat16, kind="ExternalOutput"
    )

    with tile.TileContext(nc) as tc:
        with (
            tc.tile_pool(name="idxp", bufs=1) as idxp,
            tc.tile_pool(name="gath", bufs=bufs) as gath,
            tc.tile_pool(name="fold", bufs=6) as foldp,
            tc.tile_pool(name="accp", bufs=3) as accp,
        ):
            gsb = idxp.tile([P, pl["idxw"]], mybir.dt.int16)
            nc.sync.dma_start(out=gsb[:], in_=gi.ap())
            for _rep in range(reps):  # reps>1 only for slope timing
                _build_jobs(
                    nc, bass, mybir, pl, xs, gsb, gath, foldp, accp, out,
                    queues, skip_reduce,
                )
    nc.compile()
    return nc


def _build_jobs(
    nc, bass, mybir, pl, xs, gsb, gath, foldp, accp, out, queues=1, skip_reduce=False
):
    off = 0
    for k, (w, t, cc, it) in enumerate(pl["jobs"]):
        h = it * cc  # rows per partition in the gathered tile
        g = gath.tile([P, h * D], mybir.dt.bfloat16, tag="g")
        in_ap = bass.AP(
            tensor=xs,
            offset=w * W_PAIRS * 2 * D,
            ap=[[2 * D, NW_PAIRS], [1, cc * D]],
        )
        it0 = 0
        while it0 < it:
            itn = min(MAX_IT_PER_CALL, it - it0)
            gv = g[:, it0 * cc * D : (it0 + itn) * cc * D]
            nc.gpsimd.dma_gather(
                gv.rearrange("p (i e) -> p i e", e=cc * D),
                in_ap,
                gsb[:, off + 8 * it0 : off + 8 * (it0 + itn)],
                num_idxs=P * itn,
                num_idxs_reg=P * itn,
                elem_size=cc * D,
                elem_step=2 * D,
                queue_num=k % queues,
            )
            it0 += itn
        if skip_reduce:
            nc.sync.dma_start(out=out[k * P : (k + 1) * P, :], in_=g[:, :D])
        else:
            acc = accp.tile([P, D], mybir.dt.bfloat16, tag="a")
            ha = h
            src = g
            if ha > FOLD_STOP:
                h1 = (ha + 1) // 2
                h2 = max(1, (h1 + 1) // 2)
                sbufs = [
                    foldp.tile([P, h1 * D], mybir.dt.bfloat16, tag="s1", name="s1"),
                    foldp.tile([P, h2 * D], mybir.dt.bfloat16, tag="s2", name="s2"),
                ]
                bi = 0
                while ha > FOLD_STOP:
                    hb = (ha + 1) // 2
                    dst = sbufs[bi]
                    bi ^= 1
                    # overlap-fold: rows [0,hb) vs rows [ha-hb, ha); the
                    # middle row of an odd count appears twice — free for max
                    nc.vector.tensor_tensor(
                        dst[:, : hb * D],
                        src[:, : hb * D],
                        src[:, (ha - hb) * D : ha * D],
                        op=mybir.AluOpType.max,
                    )
                    src = dst
                    ha = hb
            nc.vector.tensor_reduce(
                acc[:],
                src[:, : ha * D].rearrange("p (r d) -> p d r", d=D),
                axis=mybir.AxisListType.X,
                op=mybir.AluOpType.max,
            )
            nc.sync.dma_start(out=out[k * P : (k + 1) * P, :], in_=acc[:])
        off += 8 * it


def stage_in_maps(x, pl):
    xb = np.asarray(x)
    if xb.dtype != BF16:
        xb = xb.astype(BF16)
    starts = pl["starts"]
    in_maps = []
    for c in range(N_CORES):
        segs = pl["core_segs"][c]
        astarts = pl["core_astarts"][c]
        alens = pl["core_alens"][c]
        arows = pl["core_arows"][c]
        lens = starts[segs + 1] - starts[segs]
        # source row (in full x) for each aligned row: pad rows replicate
        # the segment's last row
        seg_of = np.repeat(np.arange(len(segs)), alens)
        within = np.arange(arows, dtype=np.int64) - astarts[seg_of]
        src = starts[segs[seg_of]] + np.minimum(within, lens[seg_of] - 1)
        xsh = np.zeros((pl["ns"], D), dtype=BF16)
        xsh[:arows] = xb[src]
        in_maps.append({"xs": xsh, "gidx": np.ascontiguousarray(pl["gidx"][c])})
    return in_maps


def assemble(core_outs, pl):
    out = np.zeros((NUM_SEGMENTS, D), dtype=np.float32)
    for c in range(N_CORES):
        r = np.asarray(core_outs[c]).reshape(-1, D).astype(np.float32)
        ss = pl["slotseg"][c]
        m = ss >= 0
        out[ss[m]] = r[m]
    return out


def kernel(x, index):
    from concourse.bass_utils import run_bass_kernel_spmd

    x = np.ascontiguousarray(np.asarray(x, dtype=np.float32))
    index = np.asarray(index)
    assert x.shape == (N_FULL, D)

    pl = _plan(index)
    nc = _build(pl)
    in_maps = stage_in_maps(x, pl)

    trace = os.environ.get("SEGKERN_TRACE", "0") == "1"
    res = run_bass_kernel_spmd(
        nc, in_maps, core_ids=list(range(N_CORES)), trace=trace
    )
    LAST_RUN_INFO.clear()
    LAST_RUN_INFO.update(
        exec_time_ns=res.exec_time_ns,
        mean_exec_time_ns=res.mean_exec_time_ns,
        trace=res.instructions_and_trace[1] if res.instructions_and_trace else None,
        profile_json=res.profile_json,
    )
    return assemble([r["out"] for r in res.results], pl)
